# revision 4
# baseline (speedup 1.0000x reference)
"""DualAttention Trainium2 kernel (nn_DualAttention_44341242364496), v4.

Reference math (per batch element, X = points[b], shape (N=4096, C=256)):
  q = X Wq^T + bq ; k = X Wk^T + bk          (N, 32)
  P = softmax(q k^T, axis=-1)                (N, N)
  v = X Wv^T + bv                            (N, 256)
  out_p = gamma * P v + X
  E = X^T X ; A = softmax(max_d(E) - E) == stable softmax(-E)
  out_c = gamma * (X A^T) + X
  out = gamma*(Pv) + gamma*(X A^T) + 2X

Distribution: 8 cores; core c handles batch b=c//2, query-row half h=c%2.

v4 structure (vs the v2 baseline at ~135us):
 - gamma folded into Wv on host; gamma*bv is a rank-1 constant added on
   the host after the gather (P(v+bv)/denom = Pv/denom + bv).  vaug's
   ones-column comes from a one-time memset, so v chunks and outc
   blocks drain from PSUM with pure copies split across ACT/DVE.
 - the +X residual of the channel branch is folded into the attnTg
   matrix (identity added to its diagonal blocks).
 - round-loop exp writes two separate tiles - ACT cols :EXPACT, DVE
   Schraudolph cols EXPACT: - so the two engines run concurrently
   (same-tile writers serialize), keeping the 64-round loop PE-bound.
 - two scores-only prelude rounds bridge the channel-chain latency so
   the PE never idles between the energy phase and the round loop.
 - last round drains per output block (pv, pv, epilogue) to shorten
   the serial tail.
 - zero-tile fp16 warmup matmuls open the PE HAM clock gate before the
   DMA-paced energy matmuls begin.  (fp32 warmups would hang HW.)
"""

import sys

sys.path.insert(0, "/opt/trn_rl_repo")

import numpy as np

import concourse.bass as bass  # noqa: F401
import concourse.mybir as mybir
import concourse.tile as tile
from concourse import bacc
from concourse.bass_utils import run_bass_kernel_spmd
from concourse.masks import make_identity

B, N, C = 4, 4096, 256
C8 = C // 8  # 32
NCORES = 8
HALF = N // 2  # 2048 query rows per core
NBLK = HALF // 128  # 16 output row blocks per core
KCH = N // 128  # 32 key chunks
P = 128
CV = C + 1  # 257: v channels + denominator ones-column

F32 = mybir.dt.float32
U16 = mybir.dt.uint16
BF16 = mybir.dt.bfloat16
F16 = mybir.dt.float16
AX = mybir.AxisListType
ALU = mybir.AluOpType
ACTF = mybir.ActivationFunctionType

# Schraudolph bf16 exp: u16 = round(s * 128*log2(e) + (16256 + c))
EXPA = 184.6649652337873
EXPB = 16250.5

_CACHE: dict = {}

import os

NWARM = int(os.environ.get("V4_NWARM", "8"))
EXPACT = int(os.environ.get("V4_EXPACT", "640"))  # cols on ACT; rest DVE

MQ = 512  # queries per macro block
NMM = HALF // MQ  # 4 macro blocks
RPM = KCH // 2  # 16 rounds per macro block
EXPDVE = 1024 - EXPACT


def _build_nc():
    nc = bacc.Bacc("TRN2", target_bir_lowering=False)

    x2T_d = nc.dram_tensor("x2T", [P, 2, N], F16, kind="ExternalInput")
    x2_d = nc.dram_tensor("x2", [P, KCH, C], F16, kind="ExternalInput")
    wqT_d = nc.dram_tensor("wqT", [P, 2, C8], F16, kind="ExternalInput")
    wkT_d = nc.dram_tensor("wkT", [P, 2, C8], F16, kind="ExternalInput")
    wvT_d = nc.dram_tensor("wvT", [P, 2, C], F16, kind="ExternalInput")
    bq_d = nc.dram_tensor("bqc4", [4 * C8, 1], F32, kind="ExternalInput")
    bk_d = nc.dram_tensor("bkc4", [4 * C8, 1], F32, kind="ExternalInput")
    gam_d = nc.dram_tensor("gam", [1, 1], F32, kind="ExternalInput")
    out_d = nc.dram_tensor("out_rows", [NBLK, P, C], BF16, kind="ExternalOutput")

    with tile.TileContext(nc) as tc:
        with (
            tc.tile_pool(name="singles", bufs=1) as singles,
            tc.tile_pool(name="persist", bufs=1) as persist,
            tc.tile_pool(name="pTa", bufs=3) as pTa,
            tc.tile_pool(name="pTd", bufs=3) as pTd,
            tc.tile_pool(name="sbout", bufs=8) as sbout,
            tc.tile_pool(name="small", bufs=16) as small,
            tc.tile_pool(name="psS", bufs=2, space="PSUM") as psS,
            tc.tile_pool(name="psO", bufs=4, space="PSUM") as psO,
        ):
            # ---------------- Phase A: loads & constants ----------------
            if NWARM:
                wz = singles.tile([P, 512], F16, tag="wz")
                nc.gpsimd.memset(wz[:], 0.0)
                warm = [
                    psO.tile([P, 512], F32, tag="o", name=f"warm{i}")[:, :512]
                    for i in range(4)
                ]
                for i in range(NWARM):
                    nc.tensor.matmul(
                        warm[i % 4], wz[:, :P], wz[:], start=True, stop=True
                    )
            gb = singles.tile([P, 1], F32, tag="gb")
            nc.scalar.dma_start(gb[:], gam_d.ap().to_broadcast([P, 1]))
            wqT = singles.tile([P, 2, C8], F16, tag="wqT")
            nc.scalar.dma_start(wqT[:], wqT_d.ap())
            wkT = singles.tile([P, 2, C8], F16, tag="wkT")
            nc.scalar.dma_start(wkT[:], wkT_d.ap())
            wvT = singles.tile([P, 2, C], F16, tag="wvT")
            nc.scalar.dma_start(wvT[:], wvT_d.ap())
            bqc4 = singles.tile([4 * C8, 1], F32, tag="bqc4")
            nc.scalar.dma_start(bqc4[:], bq_d.ap())
            bkc4 = singles.tile([4 * C8, 1], F32, tag="bkc4")
            nc.scalar.dma_start(bkc4[:], bk_d.ap())
            x2 = persist.tile([P, KCH, C], F16, tag="x2")
            x2T = persist.tile([P, 2, N], F16, tag="x2T")
            for g in range(8):
                if g == 0:
                    nc.sync.dma_start(x2[:, 0:1, :], x2_d.ap()[:, 0:1, :])
                    nc.sync.dma_start(x2[:, 1:4, :], x2_d.ap()[:, 1:4, :])
                else:
                    nc.sync.dma_start(
                        x2[:, g * 4 : (g + 1) * 4, :],
                        x2_d.ap()[:, g * 4 : (g + 1) * 4, :],
                    )
                nc.sync.dma_start(
                    x2T[:, :, g * 512 : (g + 1) * 512],
                    x2T_d.ap()[:, :, g * 512 : (g + 1) * 512],
                )
            ident = singles.tile([P, P], F32, tag="ident")
            make_identity(nc, ident[:])
            gh = singles.tile([P, 1], F32, tag="gh")
            nc.vector.tensor_scalar_mul(gh[:], gb[:], 0.5)
            # vaug ones-column (denominator source), set once
            vaug = persist.tile([P, KCH, CV], BF16, tag="vaug")
            nc.gpsimd.memset(vaug[:, :, C : C + 1], 1.0)

            # ------- Phase B: channel attention (E = X^T X, softmax) -------
            attn_n = singles.tile([P, 2, C], F32, tag="attn_n")
            attnTg = persist.tile([P, 2, C], F16, tag="attnTg")
            e_ps = [
                psO.tile([P, 512], F32, tag="o", name=f"e_{cb}")[:, :C]
                for cb in range(2)
            ]
            # kT2[32g:32g+32, 128t:128(t+1)] = k-dims of key chunk 4t+g
            kT2 = persist.tile([P, (KCH // 4) * P], F16, tag="kT2")
            # qT2: 4 replicated row strips of the core's 2048 query q-vals
            qT2 = persist.tile([P, HALF], F16, tag="qT2")

            def emit_energy(nk):
                for cb in range(2):
                    nc.tensor.matmul(
                        e_ps[cb],
                        x2[:, nk, cb * P : (cb + 1) * P],
                        x2[:, nk, :],
                        start=(nk == 0),
                        stop=(nk == KCH - 1),
                    )

            def emit_v(nk):
                # two chunks nk, nk+1 share one PSUM bank; one drain copy
                vps = psO.tile([P, 512], F32, tag="o", name=f"v_{nk}")
                for half in range(2):
                    for cc in range(2):
                        nc.tensor.matmul(
                            vps[:, half * C : (half + 1) * C],
                            x2T[:, cc, (nk + half) * P : (nk + half + 1) * P],
                            wvT[:, cc, :],
                            start=(cc == 0),
                            stop=(cc == 1),
                        )
                src = vps[:].rearrange("a (two c) -> a two c", two=2, c=C)
                if (nk // 2) % 2 == 0:
                    nc.vector.tensor_copy(vaug[:, nk : nk + 2, :C], src)
                else:
                    nc.scalar.copy(vaug[:, nk : nk + 2, :C], src)

            def emit_k(quarter):
                # 4-way column-tiled: group g -> psum partitions 32g..32g+31,
                # keys of chunks {8q+g, 8q+4+g} (256 cols per group)
                kps = psS.tile([P, 1024], F32, tag="s", name=f"k_{quarter}")
                xr = [
                    x2T[:, cc, :].rearrange(
                        "a (t four p) -> a four t p", four=4, p=P
                    )
                    for cc in range(2)
                ]
                for cc in range(2):
                    for g in range(4):
                        nc.tensor.matmul(
                            kps[g * C8 : (g + 1) * C8, :256],
                            wkT[:, cc, :],
                            xr[cc][:, g, 2 * quarter : 2 * quarter + 2, :],
                            start=(cc == 0),
                            stop=(cc == 1),
                            tile_position=(0, g * C8),
                            skip_group_check=True,
                        )
                nc.scalar.activation(
                    kT2[:, quarter * 256 : (quarter + 1) * 256],
                    kps[:, :256],
                    ACTF.Identity,
                    bias=bkc4[:],
                )

            def emit_q(seg):
                # 4 replicated row strips of q via 4-way column tiling
                qps = psS.tile([P, 1024], F32, tag="s", name=f"q_{seg}")
                for cc in range(2):
                    for g in range(4):
                        nc.tensor.matmul(
                            qps[g * C8 : (g + 1) * C8, :512],
                            wqT[:, cc, :],
                            x2T[:, cc, seg * 512 : (seg + 1) * 512],
                            start=(cc == 0),
                            stop=(cc == 1),
                            tile_position=(0, g * C8),
                            skip_group_check=True,
                        )
                nc.scalar.activation(
                    qT2[:, seg * 512 : (seg + 1) * 512],
                    qps[:, :512],
                    ACTF.Identity,
                    bias=bqc4[:],
                )

            outc_sb = persist.tile([P, NBLK, C], F32, tag="outc_sb")

            def emit_outc(blk):
                # c_ps = gamma*attn_c-part@X + 2X (residual via attnTg ident)
                c_ps = psO.tile([P, 512], F32, tag="o", name=f"c_{blk}")[:, :C]
                for dd in range(2):
                    nc.tensor.matmul(
                        c_ps,
                        x2T[:, dd, blk * P : (blk + 1) * P],
                        attnTg[:, dd, :],
                        start=(dd == 0),
                        stop=(dd == 1),
                    )
                if blk % 2 == 0:
                    nc.vector.tensor_copy(outc_sb[:, blk, :], c_ps)
                else:
                    nc.scalar.copy(outc_sb[:, blk, :], c_ps)

            # Dense DMA-paced loop: as group g of x2/x2T lands, its energy
            # chunks, v chunks, and (odd g) k/q quarter all emit.
            for g in range(8):
                for nk in range(4 * g, 4 * g + 4):
                    emit_energy(nk)
                    if nk % 2 == 0:
                        emit_v(nk)
                if g % 2 == 1:
                    emit_k((g - 1) // 2)
                    emit_q((g - 1) // 2)

            def chain(cb):
                emin = small.tile([P, 1], F32, tag="sm", name=f"emin{cb}")
                nc.vector.tensor_reduce(
                    emin[:], e_ps[cb], axis=AX.X, op=ALU.min
                )
                emq = small.tile([P, 1], F32, tag="sm", name=f"emq{cb}")
                nc.vector.tensor_scalar_mul(emq[:], emin[:], 0.25)
                us = small.tile([P, 1], F32, tag="sm", name=f"us{cb}")
                nc.scalar.activation(
                    attn_n[:, cb, :],
                    e_ps[cb],
                    ACTF.Exp,
                    bias=emq[:],
                    scale=-0.25,
                    accum_out=us[:],
                )
                rc = small.tile([P, 1], F32, tag="sm", name=f"rc{cb}")
                nc.vector.reciprocal(rc[:], us[:])
                rcg = small.tile([P, 1], F32, tag="sm", name=f"rcg{cb}")
                nc.vector.tensor_mul(rcg[:], rc[:], gh[:])
                nc.vector.tensor_scalar_mul(
                    attn_n[:, cb, :], attn_n[:, cb, :], rcg[:]
                )

            def transposes(cb):
                # attn_n[:, cb, :] -> attnTg[:, :, cb*P:(cb+1)*P]; the
                # diagonal block (dd == cb) gains +ident so the outc matmul
                # also produces the 2X residual.
                for dd in range(2):
                    t_ps = psO.tile(
                        [P, 512], F32, tag="o", name=f"t_{dd}{cb}"
                    )[:, :P]
                    nc.tensor.transpose(
                        t_ps,
                        attn_n[:, cb, dd * P : (dd + 1) * P],
                        ident[:],
                    )
                    dst = attnTg[:, dd, cb * P : (cb + 1) * P]
                    if dd == cb:
                        nc.vector.tensor_add(dst, t_ps, ident[:])
                    else:
                        nc.scalar.copy(dst, t_ps)

            # ---------------- Phase D: point attention ----------------
            # Round r covers key chunks 2r, 2r+1, in kT2 row strips {0,1}
            # (even r) or {2,3} (odd r): the two score matmuls run
            # concurrently in distinct PE row groups.  exp splits across
            # ACT (pT_a) and DVE Schraudolph (pT_d); the LAG=2 pipeline
            # keeps PE free of mid-phase semaphore stalls.
            o_ps: dict = {}
            pT_t: dict = {}

            def emit_scores(m, r):
                gbase = 2 * (r % 2)
                tt = r // 2
                s_ps = psS.tile([P, 1024], F32, tag="s", name=f"s_{m}_{r}")
                for h in range(2):
                    g = gbase + h
                    nc.tensor.matmul(
                        s_ps[:, h * 512 : (h + 1) * 512],
                        kT2[g * C8 : (g + 1) * C8, tt * P : (tt + 1) * P],
                        qT2[g * C8 : (g + 1) * C8, m * 512 : (m + 1) * 512],
                        start=True,
                        stop=True,
                        tile_position=(g * C8, 0),
                        skip_group_check=True,
                    )
                pa = pTa.tile([P, EXPACT], BF16, tag="pa", name=f"pa_{m}_{r}")
                pd = pTd.tile([P, EXPDVE], BF16, tag="pd", name=f"pd_{m}_{r}")
                nc.scalar.activation(pa[:], s_ps[:, :EXPACT], ACTF.Exp)
                nc.vector.tensor_scalar(
                    pd[:].bitcast(U16),
                    s_ps[:, EXPACT:1024],
                    EXPA,
                    EXPB,
                    op0=ALU.mult,
                    op1=ALU.add,
                )
                pT_t[(m, r)] = (pa, pd)

            def pv_slice(pa, pd, h, j):
                col = h * 512 + j * P
                if col < EXPACT:
                    return pa[:, col : col + P]
                return pd[:, col - EXPACT : col - EXPACT + P]

            def emit_pv(m, r, j_order=None):
                pa, pd = pT_t.pop((m, r))
                for h in range(2):
                    kk = 2 * r + h
                    for j in j_order or range(4):
                        nc.tensor.matmul(
                            o_ps[(m, j)],
                            pv_slice(pa, pd, h, j),
                            vaug[:, kk, :],
                            start=(kk == 0),
                            stop=(kk == KCH - 1),
                        )

            def epilogue_block(m, j, last):
                blk = m * 4 + j
                ops = o_ps.pop((m, j))
                if last:
                    osb = ops
                else:
                    osb = sbout.tile([P, CV], F32, tag="acc", name=f"osb{blk}")
                    nc.vector.tensor_copy(osb[:], ops)
                rq = small.tile([P, 1], F32, tag="sm", name=f"rq{blk}")
                nc.vector.reciprocal(rq[:], osb[:, C : C + 1])
                acc = sbout.tile([P, C], BF16, tag="acc", name=f"acc{blk}")
                if last and j >= 2:
                    nc.scalar.mul(acc[:], osb[:, :C], rq[:])
                    nc.vector.tensor_add(acc[:], acc[:], outc_sb[:, blk, :])
                else:
                    nc.vector.scalar_tensor_tensor(
                        acc[:],
                        osb[:, :C],
                        rq[:],
                        outc_sb[:, blk, :],
                        op0=ALU.mult,
                        op1=ALU.add,
                    )
                nc.sync.dma_start(out_d.ap()[blk], acc[:])

            def emit_epilogue(m):
                for j in range(4):
                    epilogue_block(m, j, last=False)

            # chain latency hides under two scores-only prelude rounds;
            # transposes + outc slot in before the first PV needs PSUM.
            chain(0)
            chain(1)
            rounds = [(m, r) for m in range(NMM) for r in range(RPM)]
            LAG = 2
            emit_scores(*rounds[0])
            emit_scores(*rounds[1])
            transposes(0)
            transposes(1)
            for blk in range(NBLK):
                emit_outc(blk)
            for i in range(LAG, len(rounds)):
                m, r = rounds[i]
                emit_scores(m, r)
                pm, pr = rounds[i - LAG]
                if pr == 0:
                    for j in range(4):
                        o_ps[(pm, j)] = psO.tile(
                            [P, 512], F32, tag="o", name=f"o_{pm}_{j}"
                        )[:, :CV]
                emit_pv(pm, pr)
                if pr == RPM - 1:
                    emit_epilogue(pm)
            # LAG drain: second-to-last round plain; last round interleaves
            # (pv h0, pv h1, epilogue) per output block to shorten the tail.
            pm, pr = rounds[-2]
            emit_pv(pm, pr)
            pm, pr = rounds[-1]
            pa, pd = pT_t.pop((pm, pr))
            for j in range(4):
                for h in range(2):
                    kk = 2 * pr + h
                    nc.tensor.matmul(
                        o_ps[(pm, j)],
                        pv_slice(pa, pd, h, j),
                        vaug[:, kk, :],
                        start=False,
                        stop=(h == 1),
                    )
                epilogue_block(pm, j, last=True)

    nc.compile()
    return nc


def _prep_core_inputs(points, core):
    b, h = core // 2, core % 2
    xb = np.asarray(points[b], dtype=np.float32)
    # own rows first, then the other half (key order is softmax-invariant
    # as long as kT and v use the same order, which they do)
    xp = np.concatenate([xb[h * HALF : (h + 1) * HALF], xb[(1 - h) * HALF : (2 - h) * HALF]])
    x2T = np.ascontiguousarray(
        (2.0 * xp).T.reshape(2, P, N).transpose(1, 0, 2)
    ).astype(np.float16)  # (128, 2, 4096)
    x2 = (2.0 * xp).reshape(KCH, P, C).transpose(1, 0, 2).astype(np.float16)
    return {"x2T": x2T, "x2": x2}


def _prep_shared_inputs(Wq, bq, Wk, bk, Wv, bv, gamma):
    g = float(np.asarray(gamma, np.float32).reshape(()))
    wqT = np.ascontiguousarray((0.5 * np.asarray(Wq, np.float32).T).reshape(2, P, C8).transpose(1, 0, 2)).astype(np.float16)
    wkT = np.ascontiguousarray((0.5 * np.asarray(Wk, np.float32).T).reshape(2, P, C8).transpose(1, 0, 2)).astype(np.float16)
    # gamma folded into Wv (x2 carries 2.0, hence 0.5)
    wvT_full = (0.5 * g) * np.asarray(Wv, np.float32).T  # (256, 256)
    wvT = np.ascontiguousarray(wvT_full.reshape(2, P, C).transpose(1, 0, 2)).astype(np.float16)
    return {
        "wqT": wqT,
        "wkT": wkT,
        "wvT": wvT,
        "bqc4": np.tile(np.asarray(bq, np.float32), 4).reshape(4 * C8, 1),
        "bkc4": np.tile(np.asarray(bk, np.float32), 4).reshape(4 * C8, 1),
        "gam": np.asarray(gamma, np.float32).reshape(1, 1),
    }


def kernel(points, Wq, bq, Wk, bk, Wv, bv, gamma, **run_kwargs):
    if "nc" not in _CACHE:
        _CACHE["nc"] = _build_nc()
    nc = _CACHE["nc"]

    shared = _prep_shared_inputs(Wq, bq, Wk, bk, Wv, bv, gamma)
    in_maps = []
    for core in range(NCORES):
        m = dict(shared)
        m.update(_prep_core_inputs(points, core))
        in_maps.append(m)

    res = run_bass_kernel_spmd(
        nc, in_maps, core_ids=list(range(NCORES)), **run_kwargs
    )
    # gamma*bv is a rank-1 constant of the output; add it on the host
    gbv = (
        np.asarray(gamma, np.float32).reshape(()) * np.asarray(bv, np.float32)
    ).reshape(1, 1, C)
    out = np.empty((B, N, C), dtype=np.float32)
    for core in range(NCORES):
        b, h = core // 2, core % 2
        out[b, h * HALF : (h + 1) * HALF] = (
            res.results[core]["out_rows"].reshape(HALF, C).astype(np.float32)
        )
    out += gbv
    if run_kwargs:
        kernel.last_results = res  # expose profile info to test harness
    return out


# revision 8
# speedup vs baseline: 1.1474x; 1.1474x over previous
"""DualAttention Trainium2 kernel (nn_DualAttention_44341242364496), v5.

Reference math (per batch element, X = points[b], shape (N=4096, C=256)):
  q = X Wq^T + bq ; k = X Wk^T + bk          (N, 32)
  P = softmax(q k^T, axis=-1)                (N, N)
  v = X Wv^T + bv                            (N, 256)
  out_p = gamma * P v + X
  E = X^T X ; A = softmax(max_d(E) - E) == stable softmax(-E)
  out_c = gamma * (X A^T) + X
  out = gamma*(Pv) + gamma*(X A^T) + 2X

Distribution: 8 cores; core c handles batch b=c//2, query-row half h=c%2.

v5 structure (vs the v2 baseline at ~135us):
 - super-rounds of 4 key chunks: the four K=32 score matmuls run
   concurrently in the four PE row strips, each into its own single-
   bank PSUM pool (row tiles must never share a bank), amortizing the
   score->PV weight-slot transition over 16 PV matmuls.  The exp of
   round r only has to finish before round r+1's scores, a full
   super-round of slack, so the loop is PE-bound.
 - exp splits by strip: ACT takes strips 0-1 (tile pa), DVE
   Schraudolph strips 2-3 (tile pd) - separate tiles, concurrent
   engines.
 - gamma folded into Wv on host; gamma*bv is a rank-1 constant added
   on the host after the gather.  vaug's ones-column comes from a
   memset, so v chunks and outc blocks drain from PSUM with pure
   copies split across ACT/DVE, two blocks per PSUM bank.
 - the +X residual of the channel branch is folded into attnTg
   (identity added to its diagonal blocks).
 - epilogues split into urgent PSUM-freeing copies (spread ACT/DVE)
   and deferred per-block finishes metered one per two rounds.
 - zero-tile fp16 warmup matmuls open the PE HAM clock gate before the
   DMA-paced energy matmuls begin.  (fp32 warmups would hang HW.)
"""

import sys

sys.path.insert(0, "/opt/trn_rl_repo")

import numpy as np

import concourse.bass as bass  # noqa: F401
import concourse.mybir as mybir
import concourse.tile as tile
from concourse import bacc
from concourse.bass_utils import run_bass_kernel_spmd
from concourse.masks import make_identity

B, N, C = 4, 4096, 256
C8 = C // 8  # 32
NCORES = 8
HALF = N // 2  # 2048 query rows per core
NBLK = HALF // 128  # 16 output row blocks per core
KCH = N // 128  # 32 key chunks
P = 128
CV = C + 1  # 257: v channels + denominator ones-column

F32 = mybir.dt.float32
U16 = mybir.dt.uint16
BF16 = mybir.dt.bfloat16
F16 = mybir.dt.float16
AX = mybir.AxisListType
ALU = mybir.AluOpType
ACTF = mybir.ActivationFunctionType

# Schraudolph bf16 exp: u16 = round(s * 128*log2(e) + (16256 + c))
EXPA = 184.6649652337873
EXPB = 16250.5

_CACHE: dict = {}

import os

NWARM = int(os.environ.get("V5_NWARM", "8"))

MQ = 512  # queries per macro block
NMM = HALF // MQ  # 4 macro blocks
RPM = KCH // 4  # 8 super-rounds (4 key chunks each) per macro block


def _build_nc():
    nc = bacc.Bacc("TRN2", target_bir_lowering=False)

    x2T_d = nc.dram_tensor("x2T", [P, 2, N], F16, kind="ExternalInput")
    x2_d = nc.dram_tensor("x2", [P, KCH, C], F16, kind="ExternalInput")
    wqT_d = nc.dram_tensor("wqT", [P, 2, C8], F16, kind="ExternalInput")
    wkT_d = nc.dram_tensor("wkT", [P, 2, C8], F16, kind="ExternalInput")
    wvT_d = nc.dram_tensor("wvT", [P, 2, C], F16, kind="ExternalInput")
    bq_d = nc.dram_tensor("bqc4", [4 * C8, 1], F32, kind="ExternalInput")
    bk_d = nc.dram_tensor("bkc4", [4 * C8, 1], F32, kind="ExternalInput")
    gam_d = nc.dram_tensor("gam", [1, 1], F32, kind="ExternalInput")
    out_d = nc.dram_tensor("out_rows", [NBLK, P, C], BF16, kind="ExternalOutput")

    with tile.TileContext(nc) as tc:
        with (
            tc.tile_pool(name="singles", bufs=1) as singles,
            tc.tile_pool(name="persist", bufs=1) as persist,
            tc.tile_pool(name="pTa", bufs=2) as pTa,
            tc.tile_pool(name="pTd", bufs=2) as pTd,
            tc.tile_pool(name="sbout", bufs=8) as sbout,
            tc.tile_pool(name="small", bufs=16) as small,
            tc.tile_pool(name="ps0", bufs=1, space="PSUM") as ps0,
            tc.tile_pool(name="ps1", bufs=1, space="PSUM") as ps1,
            tc.tile_pool(name="ps2", bufs=1, space="PSUM") as ps2,
            tc.tile_pool(name="ps3", bufs=1, space="PSUM") as ps3,
            tc.tile_pool(name="psO", bufs=4, space="PSUM") as psO,
        ):
            strip_pool = [ps0, ps1, ps2, ps3]
            # ---------------- Phase A: loads & constants ----------------
            if NWARM:
                wz = singles.tile([P, 512], F16, tag="wz")
                nc.gpsimd.memset(wz[:], 0.0)
                warm = [
                    psO.tile([P, 512], F32, tag="o", name=f"warm{i}")[:, :512]
                    for i in range(4)
                ]
                for i in range(NWARM):
                    nc.tensor.matmul(
                        warm[i % 4], wz[:, :P], wz[:], start=True, stop=True
                    )
            gb = singles.tile([P, 1], F32, tag="gb")
            nc.scalar.dma_start(gb[:], gam_d.ap().to_broadcast([P, 1]))
            wqT = singles.tile([P, 2, C8], F16, tag="wqT")
            nc.scalar.dma_start(wqT[:], wqT_d.ap())
            wkT = singles.tile([P, 2, C8], F16, tag="wkT")
            nc.scalar.dma_start(wkT[:], wkT_d.ap())
            wvT = singles.tile([P, 2, C], F16, tag="wvT")
            nc.scalar.dma_start(wvT[:], wvT_d.ap())
            bqc4 = singles.tile([4 * C8, 1], F32, tag="bqc4")
            nc.scalar.dma_start(bqc4[:], bq_d.ap())
            bkc4 = singles.tile([4 * C8, 1], F32, tag="bkc4")
            nc.scalar.dma_start(bkc4[:], bk_d.ap())
            x2 = persist.tile([P, KCH, C], F16, tag="x2")
            x2T = persist.tile([P, 2, N], F16, tag="x2T")
            for g in range(8):
                if g == 0:
                    nc.sync.dma_start(x2[:, 0:1, :], x2_d.ap()[:, 0:1, :])
                    nc.sync.dma_start(x2[:, 1:4, :], x2_d.ap()[:, 1:4, :])
                else:
                    nc.sync.dma_start(
                        x2[:, g * 4 : (g + 1) * 4, :],
                        x2_d.ap()[:, g * 4 : (g + 1) * 4, :],
                    )
                nc.sync.dma_start(
                    x2T[:, :, g * 512 : (g + 1) * 512],
                    x2T_d.ap()[:, :, g * 512 : (g + 1) * 512],
                )
            ident = singles.tile([P, P], F32, tag="ident")
            make_identity(nc, ident[:])
            gh = singles.tile([P, 1], F32, tag="gh")
            nc.vector.tensor_scalar_mul(gh[:], gb[:], 0.5)
            # vaug ones-column (denominator source), set once
            vaug = persist.tile([P, KCH, CV], BF16, tag="vaug")
            nc.gpsimd.memset(vaug[:, :, C : C + 1], 1.0)

            # ------- Phase B: channel attention (E = X^T X, softmax) -------
            attn_n = singles.tile([P, 2, C], F32, tag="attn_n")
            attnTg = persist.tile([P, 2, C], F16, tag="attnTg")
            e_ps = [
                psO.tile([P, 512], F32, tag="o", name=f"e_{cb}")[:, :C]
                for cb in range(2)
            ]
            # kT2[32g:32g+32, 128t:128(t+1)] = k-dims of key chunk 4t+g
            kT2 = persist.tile([P, (KCH // 4) * P], F16, tag="kT2")
            # qT2: 4 replicated row strips of the core's 2048 query q-vals
            qT2 = persist.tile([P, HALF], F16, tag="qT2")

            def emit_energy(nk):
                for cb in range(2):
                    nc.tensor.matmul(
                        e_ps[cb],
                        x2[:, nk, cb * P : (cb + 1) * P],
                        x2[:, nk, :],
                        start=(nk == 0),
                        stop=(nk == KCH - 1),
                    )

            def emit_v(nk):
                # two chunks nk, nk+1 share one PSUM bank; one drain copy
                vps = psO.tile([P, 512], F32, tag="o", name=f"v_{nk}")
                for half in range(2):
                    for cc in range(2):
                        nc.tensor.matmul(
                            vps[:, half * C : (half + 1) * C],
                            x2T[:, cc, (nk + half) * P : (nk + half + 1) * P],
                            wvT[:, cc, :],
                            start=(cc == 0),
                            stop=(cc == 1),
                        )
                src = vps[:].rearrange("a (two c) -> a two c", two=2, c=C)
                if (nk // 2) % 2 == 0:
                    nc.vector.tensor_copy(vaug[:, nk : nk + 2, :C], src)
                else:
                    nc.scalar.copy(vaug[:, nk : nk + 2, :C], src)

            def emit_k(quarter):
                # 4-way column-tiled: group g -> psum partitions 32g..32g+31,
                # keys of chunks {8q+g, 8q+4+g} (256 cols per group)
                kps = strip_pool[quarter % 4].tile(
                    [P, 512], F32, tag="s", name=f"k_{quarter}"
                )
                xr = [
                    x2T[:, cc, :].rearrange(
                        "a (t four p) -> a four t p", four=4, p=P
                    )
                    for cc in range(2)
                ]
                for cc in range(2):
                    for g in range(4):
                        nc.tensor.matmul(
                            kps[g * C8 : (g + 1) * C8, :256],
                            wkT[:, cc, :],
                            xr[cc][:, g, 2 * quarter : 2 * quarter + 2, :],
                            start=(cc == 0),
                            stop=(cc == 1),
                            tile_position=(0, g * C8),
                            skip_group_check=True,
                        )
                nc.scalar.activation(
                    kT2[:, quarter * 256 : (quarter + 1) * 256],
                    kps[:, :256],
                    ACTF.Identity,
                    bias=bkc4[:],
                )

            def emit_q(seg):
                # 4 replicated row strips of q via 4-way column tiling
                qps = strip_pool[(seg + 2) % 4].tile(
                    [P, 512], F32, tag="s", name=f"q_{seg}"
                )
                for cc in range(2):
                    for g in range(4):
                        nc.tensor.matmul(
                            qps[g * C8 : (g + 1) * C8, :512],
                            wqT[:, cc, :],
                            x2T[:, cc, seg * 512 : (seg + 1) * 512],
                            start=(cc == 0),
                            stop=(cc == 1),
                            tile_position=(0, g * C8),
                            skip_group_check=True,
                        )
                nc.scalar.activation(
                    qT2[:, seg * 512 : (seg + 1) * 512],
                    qps[:, :512],
                    ACTF.Identity,
                    bias=bqc4[:],
                )

            outc_sb = persist.tile([P, NBLK, C], F32, tag="outc_sb")

            def emit_outc(blk):
                # two blocks blk, blk+1 share one PSUM bank; one drain copy
                # c = gamma*attn_c-part@X + 2X (residual via attnTg ident)
                c_ps = psO.tile([P, 512], F32, tag="o", name=f"c_{blk}")
                for half in range(2):
                    for dd in range(2):
                        nc.tensor.matmul(
                            c_ps[:, half * C : (half + 1) * C],
                            x2T[:, dd, (blk + half) * P : (blk + half + 1) * P],
                            attnTg[:, dd, :],
                            start=(dd == 0),
                            stop=(dd == 1),
                        )
                src = c_ps[:].rearrange("a (two c) -> a two c", two=2, c=C)
                if (blk // 2) % 2 == 0:
                    nc.vector.tensor_copy(outc_sb[:, blk : blk + 2, :], src)
                else:
                    nc.scalar.copy(outc_sb[:, blk : blk + 2, :], src)

            # Dense DMA-paced loop: as group g of x2/x2T lands, its energy
            # chunks, v chunks, and (odd g) k/q quarter all emit.
            for g in range(8):
                for nk in range(4 * g, 4 * g + 4):
                    emit_energy(nk)
                    if nk % 2 == 0:
                        emit_v(nk)
                if g % 2 == 1:
                    emit_k((g - 1) // 2)
                    emit_q((g - 1) // 2)

            def chain(cb):
                emin = small.tile([P, 1], F32, tag="sm", name=f"emin{cb}")
                nc.vector.tensor_reduce(
                    emin[:], e_ps[cb], axis=AX.X, op=ALU.min
                )
                emq = small.tile([P, 1], F32, tag="sm", name=f"emq{cb}")
                nc.vector.tensor_scalar_mul(emq[:], emin[:], 0.25)
                us = small.tile([P, 1], F32, tag="sm", name=f"us{cb}")
                nc.scalar.activation(
                    attn_n[:, cb, :],
                    e_ps[cb],
                    ACTF.Exp,
                    bias=emq[:],
                    scale=-0.25,
                    accum_out=us[:],
                )
                rc = small.tile([P, 1], F32, tag="sm", name=f"rc{cb}")
                nc.vector.reciprocal(rc[:], us[:])
                rcg = small.tile([P, 1], F32, tag="sm", name=f"rcg{cb}")
                nc.vector.tensor_mul(rcg[:], rc[:], gh[:])
                nc.vector.tensor_scalar_mul(
                    attn_n[:, cb, :], attn_n[:, cb, :], rcg[:]
                )

            def transposes(cb):
                # attn_n[:, cb, :] -> attnTg[:, :, cb*P:(cb+1)*P]; the
                # diagonal block (dd == cb) gains +ident so the outc matmul
                # also produces the 2X residual.
                for dd in range(2):
                    t_ps = psO.tile(
                        [P, 512], F32, tag="o", name=f"t_{dd}{cb}"
                    )[:, :P]
                    nc.tensor.transpose(
                        t_ps,
                        attn_n[:, cb, dd * P : (dd + 1) * P],
                        ident[:],
                    )
                    dst = attnTg[:, dd, cb * P : (cb + 1) * P]
                    if dd == cb:
                        nc.vector.tensor_add(dst, t_ps, ident[:])
                    else:
                        nc.scalar.copy(dst, t_ps)

            # ---------------- Phase D: point attention ----------------
            # Super-round r covers key chunks 4r..4r+3 = kT2 row strips
            # 0..3 at column r.  Four concurrent K=32 score matmuls, one
            # per strip, each into its own single-bank pool.  LAG=1: round
            # r's PE slot runs PV of round r-1 while exp(r) proceeds.
            o_ps: dict = {}
            pT_t: dict = {}
            pending: list = []

            def emit_scores(m, r):
                sp = [
                    strip_pool[g].tile([P, 512], F32, tag="s", name=f"s{g}_{m}_{r}")
                    for g in range(4)
                ]
                for g in range(4):
                    nc.tensor.matmul(
                        sp[g],
                        kT2[g * C8 : (g + 1) * C8, r * P : (r + 1) * P],
                        qT2[g * C8 : (g + 1) * C8, m * 512 : (m + 1) * 512],
                        start=True,
                        stop=True,
                        tile_position=(g * C8, 0),
                        skip_group_check=True,
                    )
                pa = pTa.tile([P, 1024], BF16, tag="pa", name=f"pa_{m}_{r}")
                pd = pTd.tile([P, 1024], BF16, tag="pd", name=f"pd_{m}_{r}")
                for h in range(2):
                    nc.scalar.activation(
                        pa[:, h * 512 : (h + 1) * 512], sp[h][:], ACTF.Exp
                    )
                    nc.vector.tensor_scalar(
                        pd[:, h * 512 : (h + 1) * 512].bitcast(U16),
                        sp[2 + h][:],
                        EXPA,
                        EXPB,
                        op0=ALU.mult,
                        op1=ALU.add,
                    )
                pT_t[(m, r)] = (pa, pd)

            def pv_slice(pa, pd, h, j):
                t = pa if h < 2 else pd
                return t[:, (h % 2) * 512 + j * P : (h % 2) * 512 + (j + 1) * P]

            def emit_pv(m, r):
                pa, pd = pT_t.pop((m, r))
                for h in range(4):
                    kk = 4 * r + h
                    for j in range(4):
                        nc.tensor.matmul(
                            o_ps[(m, j)],
                            pv_slice(pa, pd, h, j),
                            vaug[:, kk, :],
                            start=(kk == 0),
                            stop=(kk == KCH - 1),
                        )

            def finish_block(m, j, osb):
                blk = m * 4 + j
                rq = small.tile([P, 1], F32, tag="sm", name=f"rq{blk}")
                nc.vector.reciprocal(rq[:], osb[:, C : C + 1])
                acc = sbout.tile([P, C], BF16, tag="acc", name=f"acc{blk}")
                nc.vector.scalar_tensor_tensor(
                    acc[:],
                    osb[:, :C],
                    rq[:],
                    outc_sb[:, blk, :],
                    op0=ALU.mult,
                    op1=ALU.add,
                )
                nc.sync.dma_start(out_d.ap()[blk], acc[:])

            pending_copies: list = []

            def emit_copy(m, j):
                blk = m * 4 + j
                ops = o_ps.pop((m, j))
                osb = sbout.tile([P, CV], F32, tag="acc", name=f"osb{blk}")
                if j % 2 == 0:
                    nc.vector.tensor_copy(osb[:], ops)
                else:
                    nc.scalar.copy(osb[:], ops)
                pending.append((m, j, osb))

            def emit_epilogue(m):
                # urgent PSUM-freeing copies (split ACT/DVE): two now, two
                # at the top of the next round; finishes are deferred and
                # metered by the round loop.
                emit_copy(m, 0)
                emit_copy(m, 1)
                pending_copies.extend([(m, 2), (m, 3)])

            # chain latency hides under the first scores; transposes + outc
            # pairs run before the first PV needs PSUM banks.
            chain(0)
            chain(1)
            rounds = [(m, r) for m in range(NMM) for r in range(RPM)]
            emit_scores(*rounds[0])
            transposes(0)
            transposes(1)
            for blk in range(0, NBLK, 2):
                emit_outc(blk)
            for i in range(1, len(rounds)):
                m, r = rounds[i]
                emit_scores(m, r)
                while pending_copies:
                    cm, cj = pending_copies.pop(0)
                    emit_copy(cm, cj)
                pm, pr = rounds[i - 1]
                if pr == 0:
                    for j in range(4):
                        o_ps[(pm, j)] = psO.tile(
                            [P, 512], F32, tag="o", name=f"o_{pm}_{j}"
                        )[:, :CV]
                emit_pv(pm, pr)
                if pr == RPM - 1:
                    emit_epilogue(pm)
                if pending and i % 2 == 0:
                    fm, fj, osb = pending.pop(0)
                    finish_block(fm, fj, osb)
            # drain: last round interleaves (pv x4, finish) per block
            pm, pr = rounds[-1]
            pa, pd = pT_t.pop((pm, pr))
            for j in range(4):
                for h in range(4):
                    kk = 4 * pr + h
                    nc.tensor.matmul(
                        o_ps[(pm, j)],
                        pv_slice(pa, pd, h, j),
                        vaug[:, kk, :],
                        start=False,
                        stop=(h == 3),
                    )
                finish_block(pm, j, o_ps.pop((pm, j)))
            for fm, fj, osb in pending:
                finish_block(fm, fj, osb)

    nc.compile()
    return nc


def _prep_core_inputs(points, core):
    b, h = core // 2, core % 2
    xb = np.asarray(points[b], dtype=np.float32)
    # own rows first, then the other half (key order is softmax-invariant
    # as long as kT and v use the same order, which they do)
    xp = np.concatenate([xb[h * HALF : (h + 1) * HALF], xb[(1 - h) * HALF : (2 - h) * HALF]])
    x2T = np.ascontiguousarray(
        (2.0 * xp).T.reshape(2, P, N).transpose(1, 0, 2)
    ).astype(np.float16)  # (128, 2, 4096)
    x2 = (2.0 * xp).reshape(KCH, P, C).transpose(1, 0, 2).astype(np.float16)
    return {"x2T": x2T, "x2": x2}


def _prep_shared_inputs(Wq, bq, Wk, bk, Wv, bv, gamma):
    g = float(np.asarray(gamma, np.float32).reshape(()))
    wqT = np.ascontiguousarray((0.5 * np.asarray(Wq, np.float32).T).reshape(2, P, C8).transpose(1, 0, 2)).astype(np.float16)
    wkT = np.ascontiguousarray((0.5 * np.asarray(Wk, np.float32).T).reshape(2, P, C8).transpose(1, 0, 2)).astype(np.float16)
    # gamma folded into Wv (x2 carries 2.0, hence 0.5)
    wvT_full = (0.5 * g) * np.asarray(Wv, np.float32).T  # (256, 256)
    wvT = np.ascontiguousarray(wvT_full.reshape(2, P, C).transpose(1, 0, 2)).astype(np.float16)
    return {
        "wqT": wqT,
        "wkT": wkT,
        "wvT": wvT,
        "bqc4": np.tile(np.asarray(bq, np.float32), 4).reshape(4 * C8, 1),
        "bkc4": np.tile(np.asarray(bk, np.float32), 4).reshape(4 * C8, 1),
        "gam": np.asarray(gamma, np.float32).reshape(1, 1),
    }


def kernel(points, Wq, bq, Wk, bk, Wv, bv, gamma, **run_kwargs):
    if "nc" not in _CACHE:
        _CACHE["nc"] = _build_nc()
    nc = _CACHE["nc"]

    shared = _prep_shared_inputs(Wq, bq, Wk, bk, Wv, bv, gamma)
    in_maps = []
    for core in range(NCORES):
        m = dict(shared)
        m.update(_prep_core_inputs(points, core))
        in_maps.append(m)

    res = run_bass_kernel_spmd(
        nc, in_maps, core_ids=list(range(NCORES)), **run_kwargs
    )
    # gamma*bv is a rank-1 constant of the output; add it on the host
    gbv = (
        np.asarray(gamma, np.float32).reshape(()) * np.asarray(bv, np.float32)
    ).reshape(1, 1, C)
    out = np.empty((B, N, C), dtype=np.float32)
    for core in range(NCORES):
        b, h = core // 2, core % 2
        out[b, h * HALF : (h + 1) * HALF] = (
            res.results[core]["out_rows"].reshape(HALF, C).astype(np.float32)
        )
    out += gbv
    if run_kwargs:
        kernel.last_results = res  # expose profile info to test harness
    return out


# revision 9
# speedup vs baseline: 1.1773x; 1.0260x over previous
"""DualAttention Trainium2 kernel (nn_DualAttention_44341242364496), v5.

Reference math (per batch element, X = points[b], shape (N=4096, C=256)):
  q = X Wq^T + bq ; k = X Wk^T + bk          (N, 32)
  P = softmax(q k^T, axis=-1)                (N, N)
  v = X Wv^T + bv                            (N, 256)
  out_p = gamma * P v + X
  E = X^T X ; A = softmax(max_d(E) - E) == stable softmax(-E)
  out_c = gamma * (X A^T) + X
  out = gamma*(Pv) + gamma*(X A^T) + 2X

Distribution: 8 cores; core c handles batch b=c//2, query-row half h=c%2.

v5 structure (vs the v2 baseline at ~135us):
 - super-rounds of 4 key chunks: the four K=32 score matmuls run
   concurrently in the four PE row strips, each into its own single-
   bank PSUM pool (row tiles must never share a bank), amortizing the
   score->PV weight-slot transition over 16 PV matmuls.  The exp of
   round r only has to finish before round r+1's scores, a full
   super-round of slack, so the loop is PE-bound.
 - exp splits by strip: ACT takes strips 0-1 (tile pa), DVE
   Schraudolph strips 2-3 (tile pd) - separate tiles, concurrent
   engines.
 - gamma folded into Wv on host; gamma*bv is a rank-1 constant added
   on the host after the gather.  vaug's ones-column comes from a
   memset, so v chunks and outc blocks drain from PSUM with pure
   copies split across ACT/DVE, two blocks per PSUM bank.
 - the +X residual of the channel branch is folded into attnTg
   (identity added to its diagonal blocks).
 - epilogues split into urgent PSUM-freeing copies (spread ACT/DVE)
   and deferred per-block finishes metered one per two rounds.
 - zero-tile fp16 warmup matmuls open the PE HAM clock gate before the
   DMA-paced energy matmuls begin.  (fp32 warmups would hang HW.)
"""

import sys

sys.path.insert(0, "/opt/trn_rl_repo")

import numpy as np

import concourse.bass as bass  # noqa: F401
import concourse.mybir as mybir
import concourse.tile as tile
from concourse import bacc
from concourse.bass_utils import run_bass_kernel_spmd
from concourse.masks import make_identity

B, N, C = 4, 4096, 256
C8 = C // 8  # 32
NCORES = 8
HALF = N // 2  # 2048 query rows per core
NBLK = HALF // 128  # 16 output row blocks per core
KCH = N // 128  # 32 key chunks
P = 128
CV = C + 1  # 257: v channels + denominator ones-column

F32 = mybir.dt.float32
U16 = mybir.dt.uint16
BF16 = mybir.dt.bfloat16
F16 = mybir.dt.float16
AX = mybir.AxisListType
ALU = mybir.AluOpType
ACTF = mybir.ActivationFunctionType

# Schraudolph bf16 exp: u16 = round(s * 128*log2(e) + (16256 + c))
EXPA = 184.6649652337873
EXPB = 16250.5

_CACHE: dict = {}

import os

NWARM = int(os.environ.get("V5_NWARM", "13"))

MQ = 512  # queries per macro block
NMM = HALF // MQ  # 4 macro blocks
RPM = KCH // 4  # 8 super-rounds (4 key chunks each) per macro block


def _build_nc():
    nc = bacc.Bacc("TRN2", target_bir_lowering=False)

    x2T_d = nc.dram_tensor("x2T", [P, 2, N], F16, kind="ExternalInput")
    x2_d = nc.dram_tensor("x2", [P, KCH, C], F16, kind="ExternalInput")
    wqT_d = nc.dram_tensor("wqT", [P, 2, C8], F16, kind="ExternalInput")
    wkT_d = nc.dram_tensor("wkT", [P, 2, C8], F16, kind="ExternalInput")
    wvT_d = nc.dram_tensor("wvT", [P, 2, C], F16, kind="ExternalInput")
    bq_d = nc.dram_tensor("bqc4", [4 * C8, 1], F32, kind="ExternalInput")
    bk_d = nc.dram_tensor("bkc4", [4 * C8, 1], F32, kind="ExternalInput")
    gam_d = nc.dram_tensor("gam", [1, 1], F32, kind="ExternalInput")
    out_d = nc.dram_tensor("out_rows", [NBLK, P, C], BF16, kind="ExternalOutput")

    with tile.TileContext(nc) as tc:
        with (
            tc.tile_pool(name="singles", bufs=1) as singles,
            tc.tile_pool(name="persist", bufs=1) as persist,
            tc.tile_pool(name="pTa", bufs=2) as pTa,
            tc.tile_pool(name="pTd", bufs=2) as pTd,
            tc.tile_pool(name="sbout", bufs=8) as sbout,
            tc.tile_pool(name="small", bufs=16) as small,
            tc.tile_pool(name="ps0", bufs=1, space="PSUM") as ps0,
            tc.tile_pool(name="ps1", bufs=1, space="PSUM") as ps1,
            tc.tile_pool(name="ps2", bufs=1, space="PSUM") as ps2,
            tc.tile_pool(name="ps3", bufs=1, space="PSUM") as ps3,
            tc.tile_pool(name="psO", bufs=4, space="PSUM") as psO,
        ):
            strip_pool = [ps0, ps1, ps2, ps3]
            # ---------------- Phase A: loads & constants ----------------
            if NWARM:
                wz = singles.tile([P, 512], F16, tag="wz")
                nc.gpsimd.memset(wz[:], 0.0)
                warm = [
                    psO.tile([P, 512], F32, tag="o", name=f"warm{i}")[:, :512]
                    for i in range(4)
                ]
                for i in range(NWARM):
                    nc.tensor.matmul(
                        warm[i % 4], wz[:, :P], wz[:], start=True, stop=True
                    )
            gb = singles.tile([P, 1], F32, tag="gb")
            nc.scalar.dma_start(gb[:], gam_d.ap().to_broadcast([P, 1]))
            wqT = singles.tile([P, 2, C8], F16, tag="wqT")
            nc.scalar.dma_start(wqT[:], wqT_d.ap())
            wkT = singles.tile([P, 2, C8], F16, tag="wkT")
            nc.scalar.dma_start(wkT[:], wkT_d.ap())
            wvT = singles.tile([P, 2, C], F16, tag="wvT")
            nc.scalar.dma_start(wvT[:], wvT_d.ap())
            bqc4 = singles.tile([4 * C8, 1], F32, tag="bqc4")
            nc.scalar.dma_start(bqc4[:], bq_d.ap())
            bkc4 = singles.tile([4 * C8, 1], F32, tag="bkc4")
            nc.scalar.dma_start(bkc4[:], bk_d.ap())
            x2 = persist.tile([P, KCH, C], F16, tag="x2")
            x2T = persist.tile([P, 2, N], F16, tag="x2T")
            for g in range(8):
                if g == 0:
                    nc.sync.dma_start(x2[:, 0:1, :], x2_d.ap()[:, 0:1, :])
                    nc.sync.dma_start(x2[:, 1:4, :], x2_d.ap()[:, 1:4, :])
                else:
                    nc.sync.dma_start(
                        x2[:, g * 4 : (g + 1) * 4, :],
                        x2_d.ap()[:, g * 4 : (g + 1) * 4, :],
                    )
                nc.sync.dma_start(
                    x2T[:, :, g * 512 : (g + 1) * 512],
                    x2T_d.ap()[:, :, g * 512 : (g + 1) * 512],
                )
            ident = singles.tile([P, P], F32, tag="ident")
            make_identity(nc, ident[:])
            gh = singles.tile([P, 1], F32, tag="gh")
            nc.vector.tensor_scalar_mul(gh[:], gb[:], 0.5)
            # vaug ones-column (denominator source), set once
            vaug = persist.tile([P, KCH, CV], BF16, tag="vaug")
            nc.gpsimd.memset(vaug[:, :, C : C + 1], 1.0)

            # ------- Phase B: channel attention (E = X^T X, softmax) -------
            attn_n = singles.tile([P, 2, C], F32, tag="attn_n")
            attnTg = persist.tile([P, 2, C], F16, tag="attnTg")
            e_ps = [
                psO.tile([P, 512], F32, tag="o", name=f"e_{cb}")[:, :C]
                for cb in range(2)
            ]
            # kT2[32g:32g+32, 128t:128(t+1)] = k-dims of key chunk 4t+g
            kT2 = persist.tile([P, (KCH // 4) * P], F16, tag="kT2")
            # qT2: 4 replicated row strips of the core's 2048 query q-vals
            qT2 = persist.tile([P, HALF], F16, tag="qT2")

            def emit_energy(nk):
                for cb in range(2):
                    nc.tensor.matmul(
                        e_ps[cb],
                        x2[:, nk, cb * P : (cb + 1) * P],
                        x2[:, nk, :],
                        start=(nk == 0),
                        stop=(nk == KCH - 1),
                    )

            def emit_v(nk):
                # two chunks nk, nk+1 share one PSUM bank; one drain copy
                vps = psO.tile([P, 512], F32, tag="o", name=f"v_{nk}")
                for half in range(2):
                    for cc in range(2):
                        nc.tensor.matmul(
                            vps[:, half * C : (half + 1) * C],
                            x2T[:, cc, (nk + half) * P : (nk + half + 1) * P],
                            wvT[:, cc, :],
                            start=(cc == 0),
                            stop=(cc == 1),
                        )
                src = vps[:].rearrange("a (two c) -> a two c", two=2, c=C)
                if (nk // 2) % 2 == 0:
                    nc.vector.tensor_copy(vaug[:, nk : nk + 2, :C], src)
                else:
                    nc.scalar.copy(vaug[:, nk : nk + 2, :C], src)

            def emit_k(quarter):
                # 4-way column-tiled: group g -> psum partitions 32g..32g+31,
                # keys of chunks {8q+g, 8q+4+g} (256 cols per group)
                kps = strip_pool[quarter % 4].tile(
                    [P, 512], F32, tag="s", name=f"k_{quarter}"
                )
                xr = [
                    x2T[:, cc, :].rearrange(
                        "a (t four p) -> a four t p", four=4, p=P
                    )
                    for cc in range(2)
                ]
                for cc in range(2):
                    for g in range(4):
                        nc.tensor.matmul(
                            kps[g * C8 : (g + 1) * C8, :256],
                            wkT[:, cc, :],
                            xr[cc][:, g, 2 * quarter : 2 * quarter + 2, :],
                            start=(cc == 0),
                            stop=(cc == 1),
                            tile_position=(0, g * C8),
                            skip_group_check=True,
                        )
                nc.scalar.activation(
                    kT2[:, quarter * 256 : (quarter + 1) * 256],
                    kps[:, :256],
                    ACTF.Identity,
                    bias=bkc4[:],
                )

            def emit_q(seg):
                # 4 replicated row strips of q via 4-way column tiling
                qps = strip_pool[(seg + 2) % 4].tile(
                    [P, 512], F32, tag="s", name=f"q_{seg}"
                )
                for cc in range(2):
                    for g in range(4):
                        nc.tensor.matmul(
                            qps[g * C8 : (g + 1) * C8, :512],
                            wqT[:, cc, :],
                            x2T[:, cc, seg * 512 : (seg + 1) * 512],
                            start=(cc == 0),
                            stop=(cc == 1),
                            tile_position=(0, g * C8),
                            skip_group_check=True,
                        )
                nc.scalar.activation(
                    qT2[:, seg * 512 : (seg + 1) * 512],
                    qps[:, :512],
                    ACTF.Identity,
                    bias=bqc4[:],
                )

            outc_sb = persist.tile([P, NBLK, C], F32, tag="outc_sb")

            def emit_outc(blk):
                # two blocks blk, blk+1 share one PSUM bank; one drain copy
                # c = gamma*attn_c-part@X + 2X (residual via attnTg ident)
                c_ps = psO.tile([P, 512], F32, tag="o", name=f"c_{blk}")
                for half in range(2):
                    for dd in range(2):
                        nc.tensor.matmul(
                            c_ps[:, half * C : (half + 1) * C],
                            x2T[:, dd, (blk + half) * P : (blk + half + 1) * P],
                            attnTg[:, dd, :],
                            start=(dd == 0),
                            stop=(dd == 1),
                        )
                src = c_ps[:].rearrange("a (two c) -> a two c", two=2, c=C)
                if (blk // 2) % 2 == 0:
                    nc.vector.tensor_copy(outc_sb[:, blk : blk + 2, :], src)
                else:
                    nc.scalar.copy(outc_sb[:, blk : blk + 2, :], src)

            # Dense DMA-paced loop: as group g of x2/x2T lands, its energy
            # chunks, v chunks, and (odd g) k/q quarter all emit.
            for g in range(8):
                if g % 2 == 1:
                    emit_q((g - 1) // 2)
                    emit_k((g - 1) // 2)
                for nk in range(4 * g, 4 * g + 4):
                    emit_energy(nk)
                    if nk % 2 == 0:
                        emit_v(nk)

            def chain(cb):
                emin = small.tile([P, 1], F32, tag="sm", name=f"emin{cb}")
                nc.vector.tensor_reduce(
                    emin[:], e_ps[cb], axis=AX.X, op=ALU.min
                )
                emq = small.tile([P, 1], F32, tag="sm", name=f"emq{cb}")
                nc.vector.tensor_scalar_mul(emq[:], emin[:], 0.25)
                us = small.tile([P, 1], F32, tag="sm", name=f"us{cb}")
                nc.scalar.activation(
                    attn_n[:, cb, :],
                    e_ps[cb],
                    ACTF.Exp,
                    bias=emq[:],
                    scale=-0.25,
                    accum_out=us[:],
                )
                rc = small.tile([P, 1], F32, tag="sm", name=f"rc{cb}")
                nc.vector.reciprocal(rc[:], us[:])
                rcg = small.tile([P, 1], F32, tag="sm", name=f"rcg{cb}")
                nc.vector.tensor_mul(rcg[:], rc[:], gh[:])
                nc.vector.tensor_scalar_mul(
                    attn_n[:, cb, :], attn_n[:, cb, :], rcg[:]
                )

            def transposes(cb):
                # attn_n[:, cb, :] -> attnTg[:, :, cb*P:(cb+1)*P]; the
                # diagonal block (dd == cb) gains +ident so the outc matmul
                # also produces the 2X residual.
                for dd in range(2):
                    t_ps = psO.tile(
                        [P, 512], F32, tag="o", name=f"t_{dd}{cb}"
                    )[:, :P]
                    nc.tensor.transpose(
                        t_ps,
                        attn_n[:, cb, dd * P : (dd + 1) * P],
                        ident[:],
                    )
                    dst = attnTg[:, dd, cb * P : (cb + 1) * P]
                    if dd == cb:
                        nc.vector.tensor_add(dst, t_ps, ident[:])
                    else:
                        nc.scalar.copy(dst, t_ps)

            # ---------------- Phase D: point attention ----------------
            # Super-round r covers key chunks 4r..4r+3 = kT2 row strips
            # 0..3 at column r.  Four concurrent K=32 score matmuls, one
            # per strip, each into its own single-bank pool.  LAG=1: round
            # r's PE slot runs PV of round r-1 while exp(r) proceeds.
            o_ps: dict = {}
            pT_t: dict = {}
            pending: list = []

            def emit_scores(m, r):
                sp = [
                    strip_pool[g].tile([P, 512], F32, tag="s", name=f"s{g}_{m}_{r}")
                    for g in range(4)
                ]
                for g in range(4):
                    nc.tensor.matmul(
                        sp[g],
                        kT2[g * C8 : (g + 1) * C8, r * P : (r + 1) * P],
                        qT2[g * C8 : (g + 1) * C8, m * 512 : (m + 1) * 512],
                        start=True,
                        stop=True,
                        tile_position=(g * C8, 0),
                        skip_group_check=True,
                    )
                pa = pTa.tile([P, 1024], BF16, tag="pa", name=f"pa_{m}_{r}")
                pd = pTd.tile([P, 1024], BF16, tag="pd", name=f"pd_{m}_{r}")
                for h in range(2):
                    nc.scalar.activation(
                        pa[:, h * 512 : (h + 1) * 512], sp[h][:], ACTF.Exp
                    )
                    nc.vector.tensor_scalar(
                        pd[:, h * 512 : (h + 1) * 512].bitcast(U16),
                        sp[2 + h][:],
                        EXPA,
                        EXPB,
                        op0=ALU.mult,
                        op1=ALU.add,
                    )
                pT_t[(m, r)] = (pa, pd)

            def pv_slice(pa, pd, h, j):
                t = pa if h < 2 else pd
                return t[:, (h % 2) * 512 + j * P : (h % 2) * 512 + (j + 1) * P]

            def emit_pv(m, r):
                pa, pd = pT_t.pop((m, r))
                for h in range(4):
                    kk = 4 * r + h
                    for j in range(4):
                        nc.tensor.matmul(
                            o_ps[(m, j)],
                            pv_slice(pa, pd, h, j),
                            vaug[:, kk, :],
                            start=(kk == 0),
                            stop=(kk == KCH - 1),
                        )

            def finish_block(m, j, osb):
                blk = m * 4 + j
                rq = small.tile([P, 1], F32, tag="sm", name=f"rq{blk}")
                nc.vector.reciprocal(rq[:], osb[:, C : C + 1])
                acc = sbout.tile([P, C], BF16, tag="acc", name=f"acc{blk}")
                if blk % 2 == 1:
                    nc.scalar.mul(acc[:], osb[:, :C], rq[:])
                    nc.vector.tensor_add(acc[:], acc[:], outc_sb[:, blk, :])
                else:
                    nc.vector.scalar_tensor_tensor(
                        acc[:],
                        osb[:, :C],
                        rq[:],
                        outc_sb[:, blk, :],
                        op0=ALU.mult,
                        op1=ALU.add,
                    )
                nc.sync.dma_start(out_d.ap()[blk], acc[:])

            pending_copies: list = []

            def emit_copy(m, j):
                blk = m * 4 + j
                ops = o_ps.pop((m, j))
                osb = sbout.tile([P, CV], F32, tag="acc", name=f"osb{blk}")
                if j % 2 == 0:
                    nc.vector.tensor_copy(osb[:], ops)
                else:
                    nc.scalar.copy(osb[:], ops)
                pending.append((m, j, osb))

            def emit_epilogue(m):
                # urgent PSUM-freeing copies (split ACT/DVE): two now, two
                # at the top of the next round; finishes are deferred and
                # metered by the round loop.
                emit_copy(m, 0)
                emit_copy(m, 1)
                pending_copies.extend([(m, 2), (m, 3)])

            # chain latency hides under the first scores; transposes + outc
            # pairs run before the first PV needs PSUM banks.
            chain(0)
            chain(1)
            rounds = [(m, r) for m in range(NMM) for r in range(RPM)]
            emit_scores(*rounds[0])
            for i in range(1, len(rounds)):
                m, r = rounds[i]
                emit_scores(m, r)
                while pending_copies:
                    cm, cj = pending_copies.pop(0)
                    emit_copy(cm, cj)
                pm, pr = rounds[i - 1]
                if pr == 0:
                    for j in range(4):
                        o_ps[(pm, j)] = psO.tile(
                            [P, 512], F32, tag="o", name=f"o_{pm}_{j}"
                        )[:, :CV]
                emit_pv(pm, pr)
                if pr == RPM - 1:
                    emit_epilogue(pm)
                    if pm == 0:
                        transposes(0)
                        transposes(1)
                        for blk in range(0, NBLK // 2, 2):
                            emit_outc(blk)
                    elif pm == 1:
                        for blk in range(NBLK // 2, NBLK, 2):
                            emit_outc(blk)
                if pending and i >= 12:
                    fm, fj, osb = pending.pop(0)
                    finish_block(fm, fj, osb)
            # drain: last round interleaves (pv x4, finish) per block
            pm, pr = rounds[-1]
            pa, pd = pT_t.pop((pm, pr))
            for j in range(4):
                for h in range(4):
                    kk = 4 * pr + h
                    nc.tensor.matmul(
                        o_ps[(pm, j)],
                        pv_slice(pa, pd, h, j),
                        vaug[:, kk, :],
                        start=False,
                        stop=(h == 3),
                    )
                finish_block(pm, j, o_ps.pop((pm, j)))
            for fm, fj, osb in pending:
                finish_block(fm, fj, osb)

    nc.compile()
    return nc


def _prep_core_inputs(points, core):
    b, h = core // 2, core % 2
    xb = np.asarray(points[b], dtype=np.float32)
    # own rows first, then the other half (key order is softmax-invariant
    # as long as kT and v use the same order, which they do)
    xp = np.concatenate([xb[h * HALF : (h + 1) * HALF], xb[(1 - h) * HALF : (2 - h) * HALF]])
    x2T = np.ascontiguousarray(
        (2.0 * xp).T.reshape(2, P, N).transpose(1, 0, 2)
    ).astype(np.float16)  # (128, 2, 4096)
    x2 = (2.0 * xp).reshape(KCH, P, C).transpose(1, 0, 2).astype(np.float16)
    return {"x2T": x2T, "x2": x2}


def _prep_shared_inputs(Wq, bq, Wk, bk, Wv, bv, gamma):
    g = float(np.asarray(gamma, np.float32).reshape(()))
    wqT = np.ascontiguousarray((0.5 * np.asarray(Wq, np.float32).T).reshape(2, P, C8).transpose(1, 0, 2)).astype(np.float16)
    wkT = np.ascontiguousarray((0.5 * np.asarray(Wk, np.float32).T).reshape(2, P, C8).transpose(1, 0, 2)).astype(np.float16)
    # gamma folded into Wv (x2 carries 2.0, hence 0.5)
    wvT_full = (0.5 * g) * np.asarray(Wv, np.float32).T  # (256, 256)
    wvT = np.ascontiguousarray(wvT_full.reshape(2, P, C).transpose(1, 0, 2)).astype(np.float16)
    return {
        "wqT": wqT,
        "wkT": wkT,
        "wvT": wvT,
        "bqc4": np.tile(np.asarray(bq, np.float32), 4).reshape(4 * C8, 1),
        "bkc4": np.tile(np.asarray(bk, np.float32), 4).reshape(4 * C8, 1),
        "gam": np.asarray(gamma, np.float32).reshape(1, 1),
    }


def kernel(points, Wq, bq, Wk, bk, Wv, bv, gamma, **run_kwargs):
    if "nc" not in _CACHE:
        _CACHE["nc"] = _build_nc()
    nc = _CACHE["nc"]

    shared = _prep_shared_inputs(Wq, bq, Wk, bk, Wv, bv, gamma)
    in_maps = []
    for core in range(NCORES):
        m = dict(shared)
        m.update(_prep_core_inputs(points, core))
        in_maps.append(m)

    res = run_bass_kernel_spmd(
        nc, in_maps, core_ids=list(range(NCORES)), **run_kwargs
    )
    # gamma*bv is a rank-1 constant of the output; add it on the host
    gbv = (
        np.asarray(gamma, np.float32).reshape(()) * np.asarray(bv, np.float32)
    ).reshape(1, 1, C)
    out = np.empty((B, N, C), dtype=np.float32)
    for core in range(NCORES):
        b, h = core // 2, core % 2
        out[b, h * HALF : (h + 1) * HALF] = (
            res.results[core]["out_rows"].reshape(HALF, C).astype(np.float32)
        )
    out += gbv
    if run_kwargs:
        kernel.last_results = res  # expose profile info to test harness
    return out
